# revision 29
# baseline (speedup 1.0000x reference)
"""MoE layer (E=8 experts, top-2 routing) on 8 Trainium2 NeuronCores.

Strategy: expert-parallel with sparse token dispatch. The host computes the
top-2 routing assignment (this is the sharding decision: which tokens each
core works on) and gathers, per core c, the xT columns of the tokens routed
to expert c.  Core c then computes, ON DEVICE, the full gating probabilities
for its tokens (fp32 gate matmul + top-2 softmax) and the dispatch-weighted
expert FFN output:
    out_c[t, :] = p_c(t) * (gelu(x[t] @ w1_c + b1_c) @ w2_c + b2_c)
plus its expert-load column (sum of p over its valid tokens).  The host
scatter-adds the 8 partial outputs into the combined output (each token
appears in exactly two experts' shards).

Gate weights are column-rotated per core so that dispatch column 0 is
always the core's own expert: one shared SPMD program for all 8 cores.

Layouts (no on-device transposes anywhere):
  x is passed transposed, xT [D, C].  Matmul1 (lhsT=w1 chunk [kd,128mh],
  rhs=xT chunk [kd, <=512t]) -> psum [128 hid, t] -> gelu -> hT [hid, t]
  (bf16).  Matmul2 (lhsT=hT chunk [kh, 128tok], rhs=w2 chunk [kh, 512d])
  -> psum [128 tok, 512 d] = natural [T, D] output layout; dispatch
  masking is a per-partition tensor_scalar multiply.
"""

import os

import numpy as np
import ml_dtypes

import concourse.bass as bass
import concourse.bacc as bacc
import concourse.mybir as mybir
import concourse.tile as tile
from concourse.bass_utils import run_bass_kernel_spmd

F32 = mybir.dt.float32
BF16 = mybir.dt.bfloat16
AF = mybir.ActivationFunctionType
ALU = mybir.AluOpType
AX = mybir.AxisListType

# Problem dims (hardcoded per contest contract)
B, S, DIM, HID, E, TOP_K = 4, 2048, 1024, 4096, 8, 2
N_CORES = 8
LB_W = 0.01
T_FULL = B * S  # 8192 — denominator for expert_load

NT_MAX = 512  # max tokens per FFN tile (fp32 psum bank limit)


def build_moe_bass(T, D=DIM, H=HID, with_b2=True, t_active=None):
    """Build the single-core SPMD program for T tokens (multiple of 128)."""
    assert T % 128 == 0 and D % 128 == 0 and H % 128 == 0
    if t_active is None:
        t_active = T
    KD = D // 128          # contraction chunks over model dim
    MH = H // 128          # hidden chunks
    NDT = min(512, D)      # output-dim tile for matmul2
    ND = D // NDT
    TC = T // 128          # 128-token gating chunks
    widths = [NT_MAX] * (T // NT_MAX)
    if T % NT_MAX:
        widths.append(T % NT_MAX)

    nc = bacc.Bacc("TRN2", target_bir_lowering=False, debug=False)

    NQ1 = 8 if (H // 128) % 8 == 0 else 1
    MHQ = (H // 128) // NQ1
    # x and weights arrive pre-tiled in exact SBUF layouts so every DMA
    # descriptor is a multi-KB contiguous run (host does the rearrange)
    xg_d = nc.dram_tensor("xg", [T // 128, 128, D // 128, 128], F32,
                          kind="ExternalInput").ap()
    xb_d = nc.dram_tensor("xb", [128 * (D // 128) * T], BF16,
                          kind="ExternalInput").ap()
    w1_d = nc.dram_tensor("w1", [NQ1, 128, D // 128, MHQ * 128], BF16,
                          kind="ExternalInput").ap()
    b1 = nc.dram_tensor("b1", [128, H // 128], F32, kind="ExternalInput").ap()
    w2_d = nc.dram_tensor("w2", [D // min(512, D), 128, H // 128, min(512, D)],
                          BF16, kind="ExternalInput").ap()
    b2 = (nc.dram_tensor("b2", [D], F32, kind="ExternalInput").ap()
          if with_b2 else None)
    gw = nc.dram_tensor("gw", [128, D // 128, E], F32, kind="ExternalInput").ap()
    gb = nc.dram_tensor("gb", [E], F32, kind="ExternalInput").ap()
    valid = nc.dram_tensor("valid", [128, T // 128], F32, kind="ExternalInput").ap()

    out = nc.dram_tensor("out", [T, D], F32, kind="ExternalOutput").ap()
    loads = nc.dram_tensor("loads", [E, 1], F32, kind="ExternalOutput").ap()

    with tile.TileContext(nc) as tc:
        with (
            tc.tile_pool(name="wpool", bufs=1) as wpool,
            tc.tile_pool(name="xb", bufs=2) as xbp,
            tc.tile_pool(name="xg", bufs=5) as xgp,
            tc.tile_pool(name="ht", bufs=1) as htp,
            tc.tile_pool(name="disp", bufs=3) as dpp,
            tc.tile_pool(name="gtmp", bufs=4) as gtp,
            tc.tile_pool(name="osb", bufs=2) as osp,
            tc.tile_pool(name="pA", bufs=3, space=bass.MemorySpace.PSUM) as pAp,
            tc.tile_pool(name="pB", bufs=3, space=bass.MemorySpace.PSUM) as pBp,
            tc.tile_pool(name="pG", bufs=1, space=bass.MemorySpace.PSUM) as pGp,
            tc.tile_pool(name="pL", bufs=1, space=bass.MemorySpace.PSUM) as pLp,
        ):
            # ---- small tensors + first x tile first (they gate the PE's
            # first work), then the big weight loads, split into
            # separately-waitable tiles so compute starts early ----
            gw_sb = wpool.tile([128, KD, E], F32)
            nc.sync.dma_start(gw_sb[:], gw)
            gb_sb = wpool.tile([1, E], F32)
            nc.sync.dma_start(gb_sb[:], gb.unsqueeze(0))
            valid_sb = wpool.tile([128, TC], F32)
            nc.sync.dma_start(valid_sb[:], valid)
            b1_sb = wpool.tile([128, MH], F32)
            nc.sync.dma_start(b1_sb[:], b1)
            if with_b2:
                b2_sb = wpool.tile([1, D], F32)
                nc.sync.dma_start(b2_sb[:], b2.unsqueeze(0))

            def load_x(tt, ts, NT):
                # gating x in per-128-token-chunk tiles so the very first
                # gate matmul waits on only 512 KB (chunk 0 split over 4
                # queues: it gates the PE's first instruction)
                xgs = []
                for c4 in range(NT // 128):
                    xgc = xgp.tile([128, KD, 128], F32, tag="xg")
                    ct = ts // 128 + c4
                    if tt == 0 and c4 == 0:
                        for q0 in range(0, 128, 32):
                            nc.sync.dma_start(
                                xgc[q0:q0 + 32, :, :], xg_d[ct, q0:q0 + 32]
                            )
                    else:
                        nc.sync.dma_start(xgc[:], xg_d[ct])
                    xgs.append(xgc)
                xb_t = xbp.tile([128, KD, NT], BF16, tag="xb")
                off = 128 * KD * ts
                nc.sync.dma_start(
                    xb_t[:],
                    xb_d[off:off + 128 * KD * NT]
                    .rearrange("(p k t) -> p k t", p=128, k=KD),
                )
                return xb_t, xgs

            # HAM pre-warm: keep PE busy on const data while the first x
            # chunk streams in, so real matmuls start at 2.4 GHz
            warm = pGp.tile([8, 8], F32, name="warm", tag="psg")
            wsrc = wpool.tile([128, 8], F32)
            nc.gpsimd.memset(wsrc[:], 1.0)
            for _ in range(150):
                nc.tensor.matmul(warm[:], wsrc[:], wsrc[:], start=True, stop=True)

            x_cur = load_x(0, 0, widths[0])

            # w1 in hid-slice tiles, w2 in out-dim-slice tiles: the first
            # FFN matmuls wait only on the first 2 MB slice, not all 16 MB.
            w1_q = []
            for q in range(NQ1):
                w1q = wpool.tile([128, KD, MHQ * 128], BF16, tag=f"w1q{q}")
                half = KD // 2
                nc.sync.dma_start(w1q[:, :half, :], w1_d[q, :, :half, :])
                nc.sync.dma_start(w1q[:, half:, :], w1_d[q, :, half:, :])
                w1_q.append(w1q)
            w2_h = []
            for nd in range(ND):
                w2h = wpool.tile([128, MH, NDT], BF16, tag=f"w2h{nd}")
                nc.sync.dma_start(w2h[:], w2_d[nd])
                w2_h.append(w2h)

            ones_f = wpool.tile([1, 128], F32)
            nc.gpsimd.memset(ones_f[:], 1.0)
            ones_b = wpool.tile([1, 128], BF16)
            nc.gpsimd.memset(ones_b[:], 1.0)
            ones_col = wpool.tile([128, 1], F32)
            nc.gpsimd.memset(ones_col[:], 1.0)

            pb2 = None
            if with_b2:
                # one-time broadcast of b2 across partitions: rank-1 matmul
                # ones^T @ b2 -> psum -> sbuf [128, D]
                pb2 = wpool.tile([128, D], F32)
                for nd0 in range(ND):
                    psb = pBp.tile([128, NDT], F32, tag="pB")
                    nc.tensor.matmul(
                        psb[:], ones_f[:1, :], b2_sb[:1, nd0 * NDT:(nd0 + 1) * NDT],
                        start=True, stop=True,
                    )
                    nc.vector.tensor_copy(pb2[:, nd0 * NDT:(nd0 + 1) * NDT], psb[:])

            pl = pLp.tile([E, 1], F32)  # expert-load accumulator (xT_FULL)

            ts = 0
            for tt, NT in enumerate(widths):
                CPT = NT // 128
                xb_t, xgs = x_cur
                if tt + 1 < len(widths):
                    x_cur = load_x(tt + 1, ts + NT, widths[tt + 1])

                # ---- gating for the CPT 128-token chunks of this tile ----
                disp_t = dpp.tile([128, CPT, E], F32, tag="disp")
                for c4 in range(CPT):
                    ct = ts // 128 + c4
                    psg = pGp.tile([128, E], F32, tag="psg")
                    for kd in range(KD):
                        nc.tensor.matmul(
                            psg[:],
                            xgs[c4][:, kd, :],
                            gw_sb[:, kd, :],
                            start=(kd == 0),
                            stop=False,
                        )
                    # + gate bias via rank-1 (ones^T @ gb)
                    nc.tensor.matmul(
                        psg[:], ones_f[:1, :], gb_sb[:1, :], start=False, stop=True
                    )
                    m1 = gtp.tile([128, 1], F32, tag="m1")
                    nc.vector.reduce_max(m1[:], psg[:], axis=AX.X)
                    y = gtp.tile([128, E], F32, tag="y")
                    nc.vector.tensor_scalar(y[:], psg[:], m1[:], None, op0=ALU.subtract)
                    eq = gtp.tile([128, E], F32, tag="eq")
                    nc.vector.tensor_scalar(eq[:], y[:], 0.0, None, op0=ALU.is_ge)
                    y2 = gtp.tile([128, E], F32, tag="y2")
                    # y2 = (eq * -1e30) + y   -> masks out the argmax entries
                    nc.vector.scalar_tensor_tensor(
                        y2[:], eq[:], -1e30, y[:], op0=ALU.mult, op1=ALU.add
                    )
                    m2 = gtp.tile([128, 1], F32, tag="m2")
                    nc.vector.reduce_max(m2[:], y2[:], axis=AX.X)
                    ey = gtp.tile([128, E], F32, tag="ey")
                    nc.scalar.activation(ey[:], y[:], AF.Exp)
                    e2 = gtp.tile([128, 1], F32, tag="e2")
                    nc.scalar.activation(e2[:], m2[:], AF.Exp)
                    den = gtp.tile([128, 1], F32, tag="den")
                    nc.vector.tensor_scalar(den[:], e2[:], 1.0, None, op0=ALU.add)
                    inv = gtp.tile([128, 1], F32, tag="inv")
                    nc.vector.reciprocal(inv[:], den[:])
                    mask = gtp.tile([128, E], F32, tag="mask")
                    nc.vector.tensor_scalar(mask[:], y[:], m2[:], None, op0=ALU.is_ge)
                    pm = gtp.tile([128, E], F32, tag="pm")
                    # pm = (ey * inv) * mask
                    nc.vector.scalar_tensor_tensor(
                        pm[:], ey[:], inv[:], mask[:], op0=ALU.mult, op1=ALU.mult
                    )
                    # disp = pm * valid  (zero out padded tokens)
                    nc.vector.tensor_scalar(
                        disp_t[:, c4, :], pm[:], valid_sb[:, ct:ct + 1], None,
                        op0=ALU.mult,
                    )
                    # expert load accumulation: disp^T @ ones -> [E, 1]
                    nc.tensor.matmul(
                        pl[:], disp_t[:, c4, :], ones_col[:],
                        start=(ct == 0), stop=(ct == TC - 1),
                    )

                # ---- phase A: hT = gelu(w1^T x + b1), [hid, tok] bf16 ----
                h_t = htp.tile([128, MH, NT], BF16, tag="ht")
                NT_a = NT
                if tt == len(widths) - 1 and tt > 0:
                    NT_a = max(1, t_active - ts)
                for mh0 in range(0, MH, 2):
                    # two interleaved accumulation groups: each group's
                    # first (wait-carrying) LDW hides inside the other's
                    # matmul stream, removing the per-group start stall
                    ps = [pAp.tile([128, NT_a], F32, name=f"psA{j}", tag="pA")
                          for j in range(2)]
                    for kd in range(KD):
                        for j in range(2):
                            q, mq = divmod(mh0 + j, MHQ)
                            nc.tensor.matmul(
                                ps[j][:],
                                w1_q[q][:, kd, mq * 128:(mq + 1) * 128],
                                xb_t[:, kd, :NT_a],
                                start=(kd == 0),
                                stop=(kd == KD - 1),
                            )
                    for j in range(2):
                        mh = mh0 + j
                        nc.scalar.activation(
                            h_t[:, mh, :NT_a], ps[j][:], AF.Gelu,
                            bias=b1_sb[:, mh:mh + 1]
                        )

                # ---- phase B: out[tok, d] = (hT^T w2 + b2) * disp[:, own] ----
                for c4 in range(CPT):
                    ct = ts // 128 + c4
                    ps2s = [pBp.tile([128, NDT], F32, name=f"psB{nd}", tag="pB")
                            for nd in range(ND)]
                    for kh in range(MH):
                        for nd in range(ND):
                            nc.tensor.matmul(
                                ps2s[nd][:],
                                h_t[:, kh, c4 * 128:(c4 + 1) * 128],
                                w2_h[nd][:, kh, :],
                                start=(kh == 0),
                                stop=(kh == MH - 1),
                            )
                    for nd in range(ND):
                        ps2 = ps2s[nd]
                        o_sb = osp.tile([128, NDT], F32, tag="osb")
                        if with_b2:
                            tso = gtp.tile([128, NDT], F32, tag="tso")
                            nc.vector.tensor_tensor(
                                tso[:], ps2[:], pb2[:, nd * NDT:(nd + 1) * NDT],
                                op=ALU.add,
                            )
                            nc.vector.tensor_scalar(
                                o_sb[:], tso[:], disp_t[:, c4, 0:1], None,
                                op0=ALU.mult,
                            )
                        else:
                            nc.vector.tensor_scalar(
                                o_sb[:], ps2[:], disp_t[:, c4, 0:1], None,
                                op0=ALU.mult,
                            )
                        if tt == len(widths) - 1:
                            nc.sync.dma_start(
                                out[ct * 128:ct * 128 + 64,
                                    nd * NDT:(nd + 1) * NDT],
                                o_sb[0:64, :],
                            )
                            nc.sync.dma_start(
                                out[ct * 128 + 64:(ct + 1) * 128,
                                    nd * NDT:(nd + 1) * NDT],
                                o_sb[64:128, :],
                            )
                        else:
                            nc.sync.dma_start(
                                out[ct * 128:(ct + 1) * 128,
                                    nd * NDT:(nd + 1) * NDT],
                                o_sb[:],
                            )
                ts += NT

            loads_sb = wpool.tile([E, 1], F32)
            nc.vector.tensor_copy(loads_sb[:], pl[:])
            nc.sync.dma_start(loads[:, :], loads_sb[:])

    nc.compile()
    return nc


_CACHED = {}


def _get_nc(T, D, H, with_b2=True, t_active=None):
    key = (T, D, H, with_b2, t_active)
    if key not in _CACHED:
        _CACHED[key] = build_moe_bass(T, D, H, with_b2, t_active)
    return _CACHED[key]


def _run_cores(nc, in_maps, trace, trace_kwargs):
    kwargs = {}
    if trace:
        kwargs = dict(trace=True, trace_kwargs=trace_kwargs or {})
    return run_bass_kernel_spmd(nc, in_maps, core_ids=list(range(N_CORES)), **kwargs)


def _per_core_weights(c, w1, b1, w2, b2, gate_w, gate_b):
    rot = [(c + i) % E for i in range(E)]
    D = w1.shape[1]
    H = w1.shape[2]
    KD, MH = D // 128, H // 128
    NQ1 = 8 if MH % 8 == 0 else 1
    MHQ = MH // NQ1
    NDT = min(512, D)
    ND = D // NDT
    # pre-tile weights into exact SBUF layouts (one contiguous run per
    # partition per DMA)
    w1t = (w1[c].astype(ml_dtypes.bfloat16).reshape(KD, 128, H)
           .transpose(1, 0, 2))                     # [128, KD, H]
    w1t = np.ascontiguousarray(
        np.stack([w1t[:, :, q * MHQ * 128:(q + 1) * MHQ * 128]
                  for q in range(NQ1)]))            # [NQ1,128,KD,MHQ*128]
    w2t = (w2[c].astype(ml_dtypes.bfloat16).reshape(MH, 128, D)
           .transpose(1, 0, 2))                     # [128, MH, D]
    w2t = np.ascontiguousarray(
        np.stack([w2t[:, :, nd * NDT:(nd + 1) * NDT]
                  for nd in range(ND)]))            # [ND,128,MH,NDT]
    return {
        "w1": w1t,
        "b1": np.ascontiguousarray(
            b1[c].astype(np.float32).reshape(MH, 128).T),
        "w2": w2t,
        "b2": np.ascontiguousarray(b2[c]).astype(np.float32),
        "gw": np.ascontiguousarray(
            gate_w[:, rot].astype(np.float32).reshape(KD, 128, E)
            .transpose(1, 0, 2)),
        "gb": np.ascontiguousarray(gate_b[rot]).astype(np.float32),
    }


def _prep_x(xg_cols, C):
    """xg_cols: [D, C] fp32 gathered x^T. Returns pre-tiled xg/xb arrays."""
    D = xg_cols.shape[0]
    KD = D // 128
    TC = C // 128
    v = xg_cols.reshape(KD, 128, C)
    xg_t = np.ascontiguousarray(
        v.reshape(KD, 128, TC, 128).transpose(2, 1, 0, 3))  # [TC,128,KD,128]
    xb_full = v.astype(ml_dtypes.bfloat16).transpose(1, 0, 2)  # [128, KD, C]
    widths = [NT_MAX] * (C // NT_MAX)
    if C % NT_MAX:
        widths.append(C % NT_MAX)
    blocks = []
    ts = 0
    for NT in widths:
        blocks.append(np.ascontiguousarray(xb_full[:, :, ts:ts + NT]).ravel())
        ts += NT
    xb_t = np.concatenate(blocks)
    return xg_t, xb_t


def run_moe(x, gate_w, gate_b, w1, b1, w2, b2, trace=False, trace_kwargs=None):
    """Sparse expert-parallel run. Returns ((combined, load_loss,
    expert_load), BassKernelResults)."""
    Bx, Sx, D = x.shape
    T = Bx * Sx
    H = w1.shape[2]

    xflat = x.reshape(T, D).astype(np.float32)
    # Host-side routing (sharding decision): which tokens go to which core.
    logits = xflat.astype(np.float64) @ gate_w.astype(np.float64) \
        + gate_b.astype(np.float64)
    top2 = np.argsort(-logits, axis=1)[:, :TOP_K]
    token_idx = [np.where((top2 == c).any(axis=1))[0] for c in range(E)]
    n_max = max(len(ix) for ix in token_idx)
    C = ((n_max + 127) // 128) * 128

    with_b2 = bool(np.any(b2))
    nc = _get_nc(C, D, H, with_b2, t_active=n_max)

    xT = np.ascontiguousarray(xflat.T)  # [D, T]
    in_maps = []
    for c in range(N_CORES):
        ix = token_idx[c]
        xg = np.zeros((D, C), dtype=np.float32)
        xg[:, :len(ix)] = xT[:, ix]
        vld = np.zeros(C, dtype=np.float32)
        vld[:len(ix)] = 1.0
        vld = np.ascontiguousarray(vld.reshape(C // 128, 128).T)
        m = _per_core_weights(c, w1, b1, w2, b2, gate_w, gate_b)
        m["xg"], m["xb"] = _prep_x(xg, C)
        m["valid"] = vld
        in_maps.append(m)

    res = _run_cores(nc, in_maps, trace, trace_kwargs)

    combined = np.zeros((T, D), dtype=np.float32)
    expert_load = np.zeros(E, dtype=np.float32)
    for c in range(N_CORES):
        ix = token_idx[c]
        np.add.at(combined, ix, res.results[c]["out"][:len(ix)])
        expert_load[c] = res.results[c]["loads"][0, 0] / np.float32(T)
    combined = combined.reshape(Bx, Sx, D)
    load_loss = np.float32(LB_W * np.sum((expert_load - 1.0 / E) ** 2))
    return (combined, load_loss, expert_load), res


def run_moe_dense(x, gate_w, gate_b, w1, b1, w2, b2, trace=False,
                  trace_kwargs=None):
    """Dense expert-parallel fallback: every core runs all T tokens."""
    Bx, Sx, D = x.shape
    T = Bx * Sx
    H = w1.shape[2]
    with_b2 = bool(np.any(b2))
    nc = _get_nc(T, D, H, with_b2)

    xT = np.ascontiguousarray(x.reshape(T, D).T.astype(np.float32))
    xg_t, xb_t = _prep_x(xT, T)
    ones = np.ones((128, T // 128), dtype=np.float32)

    in_maps = []
    for c in range(N_CORES):
        m = _per_core_weights(c, w1, b1, w2, b2, gate_w, gate_b)
        m["xg"] = xg_t
        m["xb"] = xb_t
        m["valid"] = ones
        in_maps.append(m)

    res = _run_cores(nc, in_maps, trace, trace_kwargs)

    combined = np.zeros((T, D), dtype=np.float32)
    for c in range(N_CORES):
        combined += res.results[c]["out"]
    combined = combined.reshape(Bx, Sx, D)
    expert_load = (res.results[0]["loads"][:, 0] / np.float32(T)).astype(np.float32)
    load_loss = np.float32(LB_W * np.sum((expert_load - 1.0 / E) ** 2))
    return (combined, load_loss, expert_load), res


def kernel(x, gate_w, gate_b, w1, b1, w2, b2):
    trace = bool(int(os.environ.get("BASS_MOE_TRACE", "0")))
    fn = run_moe_dense if os.environ.get("MOE_DENSE") else run_moe
    out, _ = fn(x, gate_w, gate_b, w1, b1, w2, b2, trace=trace)
    return out


# revision 30
# speedup vs baseline: 1.2194x; 1.2194x over previous
"""MoE layer (E=8 experts, top-2 routing) on 8 Trainium2 NeuronCores.

Strategy: expert-parallel with sparse token dispatch. The host computes the
top-2 routing assignment (this is the sharding decision: which tokens each
core works on) and gathers, per core c, the xT columns of the tokens routed
to expert c.  Core c then computes, ON DEVICE, the full gating probabilities
for its tokens (fp32 gate matmul + top-2 softmax) and the dispatch-weighted
expert FFN output:
    out_c[t, :] = p_c(t) * (gelu(x[t] @ w1_c + b1_c) @ w2_c + b2_c)
plus its expert-load column (sum of p over its valid tokens).  The host
scatter-adds the 8 partial outputs into the combined output (each token
appears in exactly two experts' shards).

Gate weights are column-rotated per core so that dispatch column 0 is
always the core's own expert: one shared SPMD program for all 8 cores.

Layouts (no on-device transposes anywhere):
  x is passed transposed, xT [D, C].  Matmul1 (lhsT=w1 chunk [kd,128mh],
  rhs=xT chunk [kd, <=512t]) -> psum [128 hid, t] -> gelu -> hT [hid, t]
  (bf16).  Matmul2 (lhsT=hT chunk [kh, 128tok], rhs=w2 chunk [kh, 512d])
  -> psum [128 tok, 512 d] = natural [T, D] output layout; dispatch
  masking is a per-partition tensor_scalar multiply.
"""

import os

import numpy as np
import ml_dtypes

import concourse.bass as bass
import concourse.bacc as bacc
import concourse.mybir as mybir
import concourse.tile as tile
from concourse.bass_utils import run_bass_kernel_spmd

F32 = mybir.dt.float32
BF16 = mybir.dt.bfloat16
AF = mybir.ActivationFunctionType
ALU = mybir.AluOpType
AX = mybir.AxisListType

# Problem dims (hardcoded per contest contract)
B, S, DIM, HID, E, TOP_K = 4, 2048, 1024, 4096, 8, 2
N_CORES = 8
LB_W = 0.01
T_FULL = B * S  # 8192 — denominator for expert_load

NT_MAX = 512  # max tokens per FFN tile (fp32 psum bank limit)


def build_moe_bass(T, D=DIM, H=HID, with_b2=True, t_active=None):
    """Build the single-core SPMD program for T tokens (multiple of 128)."""
    assert T % 128 == 0 and D % 128 == 0 and H % 128 == 0
    if t_active is None:
        t_active = T
    KD = D // 128          # contraction chunks over model dim
    MH = H // 128          # hidden chunks
    NDT = min(512, D)      # output-dim tile for matmul2
    ND = D // NDT
    TC = T // 128          # 128-token gating chunks
    widths = [NT_MAX] * (T // NT_MAX)
    if T % NT_MAX:
        widths.append(T % NT_MAX)

    nc = bacc.Bacc("TRN2", target_bir_lowering=False, debug=False)

    NQ1 = 8 if (H // 128) % 8 == 0 else 1
    MHQ = (H // 128) // NQ1
    # x and weights arrive pre-tiled in exact SBUF layouts so every DMA
    # descriptor is a multi-KB contiguous run (host does the rearrange)
    xg_d = nc.dram_tensor("xg", [T // 128, 128, D // 128, 128], F32,
                          kind="ExternalInput").ap()
    xb_d = nc.dram_tensor("xb", [128 * (D // 128) * T], BF16,
                          kind="ExternalInput").ap()
    w1_d = nc.dram_tensor("w1", [NQ1, 128, D // 128, MHQ * 128], BF16,
                          kind="ExternalInput").ap()
    b1 = nc.dram_tensor("b1", [128, H // 128], F32, kind="ExternalInput").ap()
    w2_d = nc.dram_tensor("w2", [D // min(512, D), 128, H // 128, min(512, D)],
                          BF16, kind="ExternalInput").ap()
    b2 = (nc.dram_tensor("b2", [D], F32, kind="ExternalInput").ap()
          if with_b2 else None)
    gw = nc.dram_tensor("gw", [128, D // 128, E], F32, kind="ExternalInput").ap()
    gb = nc.dram_tensor("gb", [E], F32, kind="ExternalInput").ap()
    valid = nc.dram_tensor("valid", [128, T // 128], F32, kind="ExternalInput").ap()

    out = nc.dram_tensor("out", [T, D], F32, kind="ExternalOutput").ap()
    loads = nc.dram_tensor("loads", [E, 1], F32, kind="ExternalOutput").ap()

    with tile.TileContext(nc) as tc:
        with (
            tc.tile_pool(name="wpool", bufs=1) as wpool,
            tc.tile_pool(name="xb", bufs=2) as xbp,
            tc.tile_pool(name="xg", bufs=5) as xgp,
            tc.tile_pool(name="ht", bufs=1) as htp,
            tc.tile_pool(name="disp", bufs=3) as dpp,
            tc.tile_pool(name="gtmp", bufs=4) as gtp,
            tc.tile_pool(name="osb", bufs=2) as osp,
            tc.tile_pool(name="pA", bufs=3, space=bass.MemorySpace.PSUM) as pAp,
            tc.tile_pool(name="pB", bufs=3, space=bass.MemorySpace.PSUM) as pBp,
            tc.tile_pool(name="pG", bufs=1, space=bass.MemorySpace.PSUM) as pGp,
            tc.tile_pool(name="pL", bufs=1, space=bass.MemorySpace.PSUM) as pLp,
        ):
            # ---- small tensors + first x tile first (they gate the PE's
            # first work), then the big weight loads, split into
            # separately-waitable tiles so compute starts early ----
            gw_sb = wpool.tile([128, KD, E], F32)
            nc.sync.dma_start(gw_sb[:], gw)
            gb_sb = wpool.tile([1, E], F32)
            nc.sync.dma_start(gb_sb[:], gb.unsqueeze(0))
            valid_sb = wpool.tile([128, TC], F32)
            nc.sync.dma_start(valid_sb[:], valid)
            b1_sb = wpool.tile([128, MH], F32)
            nc.sync.dma_start(b1_sb[:], b1)
            if with_b2:
                b2_sb = wpool.tile([1, D], F32)
                nc.sync.dma_start(b2_sb[:], b2.unsqueeze(0))

            def load_x(tt, ts, NT):
                # gating x in per-128-token-chunk tiles so the very first
                # gate matmul waits on only 512 KB (chunk 0 split over 4
                # queues: it gates the PE's first instruction)
                xgs = []
                for c4 in range(NT // 128):
                    xgc = xgp.tile([128, KD, 128], F32, tag="xg")
                    ct = ts // 128 + c4
                    if tt == 0 and c4 == 0:
                        for q0 in range(0, 128, 32):
                            nc.sync.dma_start(
                                xgc[q0:q0 + 32, :, :], xg_d[ct, q0:q0 + 32]
                            )
                    else:
                        nc.sync.dma_start(xgc[:], xg_d[ct])
                    xgs.append(xgc)
                xb_t = xbp.tile([128, KD, NT], BF16, tag="xb")
                off = 128 * KD * ts
                nc.sync.dma_start(
                    xb_t[:],
                    xb_d[off:off + 128 * KD * NT]
                    .rearrange("(p k t) -> p k t", p=128, k=KD),
                )
                return xb_t, xgs

            # HAM pre-warm: keep PE busy on const data while the first x
            # chunk streams in, so real matmuls start at 2.4 GHz
            warm = pGp.tile([8, 8], F32, name="warm", tag="psg")
            wsrc = wpool.tile([128, 8], F32)
            nc.gpsimd.memset(wsrc[:], 1.0)
            for _ in range(60):
                nc.tensor.matmul(warm[:], wsrc[:], wsrc[:], start=True, stop=True)

            x_cur = load_x(0, 0, widths[0])

            # w1 in hid-slice tiles, w2 in out-dim-slice tiles: the first
            # FFN matmuls wait only on the first 2 MB slice, not all 16 MB.
            w1_q = []
            for q in range(NQ1):
                w1q = wpool.tile([128, KD, MHQ * 128], BF16, tag=f"w1q{q}")
                nc.sync.dma_start(w1q[:], w1_d[q])
                w1_q.append(w1q)
            w2_h = []
            for nd in range(ND):
                w2h = wpool.tile([128, MH, NDT], BF16, tag=f"w2h{nd}")
                nc.sync.dma_start(w2h[:], w2_d[nd])
                w2_h.append(w2h)

            ones_f = wpool.tile([1, 128], F32)
            nc.gpsimd.memset(ones_f[:], 1.0)
            ones_b = wpool.tile([1, 128], BF16)
            nc.gpsimd.memset(ones_b[:], 1.0)
            ones_col = wpool.tile([128, 1], F32)
            nc.gpsimd.memset(ones_col[:], 1.0)

            pb2 = None
            if with_b2:
                # one-time broadcast of b2 across partitions: rank-1 matmul
                # ones^T @ b2 -> psum -> sbuf [128, D]
                pb2 = wpool.tile([128, D], F32)
                for nd0 in range(ND):
                    psb = pBp.tile([128, NDT], F32, tag="pB")
                    nc.tensor.matmul(
                        psb[:], ones_f[:1, :], b2_sb[:1, nd0 * NDT:(nd0 + 1) * NDT],
                        start=True, stop=True,
                    )
                    nc.vector.tensor_copy(pb2[:, nd0 * NDT:(nd0 + 1) * NDT], psb[:])

            pl = pLp.tile([E, 1], F32)  # expert-load accumulator (xT_FULL)

            ts = 0
            for tt, NT in enumerate(widths):
                CPT = NT // 128
                xb_t, xgs = x_cur
                if tt + 1 < len(widths):
                    x_cur = load_x(tt + 1, ts + NT, widths[tt + 1])

                # ---- gating for the CPT 128-token chunks of this tile ----
                disp_t = dpp.tile([128, CPT, E], F32, tag="disp")
                for c4 in range(CPT):
                    ct = ts // 128 + c4
                    psg = pGp.tile([128, E], F32, tag="psg")
                    for kd in range(KD):
                        nc.tensor.matmul(
                            psg[:],
                            xgs[c4][:, kd, :],
                            gw_sb[:, kd, :],
                            start=(kd == 0),
                            stop=False,
                        )
                    # + gate bias via rank-1 (ones^T @ gb)
                    nc.tensor.matmul(
                        psg[:], ones_f[:1, :], gb_sb[:1, :], start=False, stop=True
                    )
                    m1 = gtp.tile([128, 1], F32, tag="m1")
                    nc.vector.reduce_max(m1[:], psg[:], axis=AX.X)
                    y = gtp.tile([128, E], F32, tag="y")
                    nc.vector.tensor_scalar(y[:], psg[:], m1[:], None, op0=ALU.subtract)
                    eq = gtp.tile([128, E], F32, tag="eq")
                    nc.vector.tensor_scalar(eq[:], y[:], 0.0, None, op0=ALU.is_ge)
                    y2 = gtp.tile([128, E], F32, tag="y2")
                    # y2 = (eq * -1e30) + y   -> masks out the argmax entries
                    nc.vector.scalar_tensor_tensor(
                        y2[:], eq[:], -1e30, y[:], op0=ALU.mult, op1=ALU.add
                    )
                    m2 = gtp.tile([128, 1], F32, tag="m2")
                    nc.vector.reduce_max(m2[:], y2[:], axis=AX.X)
                    ey = gtp.tile([128, E], F32, tag="ey")
                    nc.scalar.activation(ey[:], y[:], AF.Exp)
                    e2 = gtp.tile([128, 1], F32, tag="e2")
                    nc.scalar.activation(e2[:], m2[:], AF.Exp)
                    den = gtp.tile([128, 1], F32, tag="den")
                    nc.vector.tensor_scalar(den[:], e2[:], 1.0, None, op0=ALU.add)
                    inv = gtp.tile([128, 1], F32, tag="inv")
                    nc.vector.reciprocal(inv[:], den[:])
                    mask = gtp.tile([128, E], F32, tag="mask")
                    nc.vector.tensor_scalar(mask[:], y[:], m2[:], None, op0=ALU.is_ge)
                    pm = gtp.tile([128, E], F32, tag="pm")
                    # pm = (ey * inv) * mask
                    nc.vector.scalar_tensor_tensor(
                        pm[:], ey[:], inv[:], mask[:], op0=ALU.mult, op1=ALU.mult
                    )
                    # disp = pm * valid  (zero out padded tokens)
                    nc.vector.tensor_scalar(
                        disp_t[:, c4, :], pm[:], valid_sb[:, ct:ct + 1], None,
                        op0=ALU.mult,
                    )
                    # expert load accumulation: disp^T @ ones -> [E, 1]
                    nc.tensor.matmul(
                        pl[:], disp_t[:, c4, :], ones_col[:],
                        start=(ct == 0), stop=(ct == TC - 1),
                    )

                # ---- phase A: hT = gelu(w1^T x + b1), [hid, tok] bf16 ----
                h_t = htp.tile([128, MH, NT], BF16, tag="ht")
                NT_a = NT
                if tt == len(widths) - 1 and tt > 0:
                    NT_a = max(1, t_active - ts)
                for mh0 in range(0, MH, 2):
                    # two interleaved accumulation groups: each group's
                    # first (wait-carrying) LDW hides inside the other's
                    # matmul stream, removing the per-group start stall
                    ps = [pAp.tile([128, NT_a], F32, name=f"psA{j}", tag="pA")
                          for j in range(2)]
                    for kd in range(KD):
                        for j in range(2):
                            q, mq = divmod(mh0 + j, MHQ)
                            nc.tensor.matmul(
                                ps[j][:],
                                w1_q[q][:, kd, mq * 128:(mq + 1) * 128],
                                xb_t[:, kd, :NT_a],
                                start=(kd == 0),
                                stop=(kd == KD - 1),
                            )
                    for j in range(2):
                        mh = mh0 + j
                        nc.scalar.activation(
                            h_t[:, mh, :NT_a], ps[j][:], AF.Gelu,
                            bias=b1_sb[:, mh:mh + 1]
                        )

                # ---- phase B: out[tok, d] = (hT^T w2 + b2) * disp[:, own] ----
                for c4 in range(CPT):
                    ct = ts // 128 + c4
                    ps2s = [pBp.tile([128, NDT], F32, name=f"psB{nd}", tag="pB")
                            for nd in range(ND)]
                    for kh in range(MH):
                        for nd in range(ND):
                            nc.tensor.matmul(
                                ps2s[nd][:],
                                h_t[:, kh, c4 * 128:(c4 + 1) * 128],
                                w2_h[nd][:, kh, :],
                                start=(kh == 0),
                                stop=(kh == MH - 1),
                            )
                    for nd in range(ND):
                        ps2 = ps2s[nd]
                        o_sb = osp.tile([128, NDT], F32, tag="osb")
                        if with_b2:
                            tso = gtp.tile([128, NDT], F32, tag="tso")
                            nc.vector.tensor_tensor(
                                tso[:], ps2[:], pb2[:, nd * NDT:(nd + 1) * NDT],
                                op=ALU.add,
                            )
                            nc.vector.tensor_scalar(
                                o_sb[:], tso[:], disp_t[:, c4, 0:1], None,
                                op0=ALU.mult,
                            )
                        else:
                            nc.vector.tensor_scalar(
                                o_sb[:], ps2[:], disp_t[:, c4, 0:1], None,
                                op0=ALU.mult,
                            )
                        if tt == len(widths) - 1:
                            nc.sync.dma_start(
                                out[ct * 128:ct * 128 + 64,
                                    nd * NDT:(nd + 1) * NDT],
                                o_sb[0:64, :],
                            )
                            nc.sync.dma_start(
                                out[ct * 128 + 64:(ct + 1) * 128,
                                    nd * NDT:(nd + 1) * NDT],
                                o_sb[64:128, :],
                            )
                        else:
                            nc.sync.dma_start(
                                out[ct * 128:(ct + 1) * 128,
                                    nd * NDT:(nd + 1) * NDT],
                                o_sb[:],
                            )
                ts += NT

            loads_sb = wpool.tile([E, 1], F32)
            nc.vector.tensor_copy(loads_sb[:], pl[:])
            nc.sync.dma_start(loads[:, :], loads_sb[:])

    nc.compile()
    return nc


_CACHED = {}


def _get_nc(T, D, H, with_b2=True, t_active=None):
    key = (T, D, H, with_b2, t_active)
    if key not in _CACHED:
        _CACHED[key] = build_moe_bass(T, D, H, with_b2, t_active)
    return _CACHED[key]


def _run_cores(nc, in_maps, trace, trace_kwargs):
    kwargs = {}
    if trace:
        kwargs = dict(trace=True, trace_kwargs=trace_kwargs or {})
    return run_bass_kernel_spmd(nc, in_maps, core_ids=list(range(N_CORES)), **kwargs)


def _per_core_weights(c, w1, b1, w2, b2, gate_w, gate_b):
    rot = [(c + i) % E for i in range(E)]
    D = w1.shape[1]
    H = w1.shape[2]
    KD, MH = D // 128, H // 128
    NQ1 = 8 if MH % 8 == 0 else 1
    MHQ = MH // NQ1
    NDT = min(512, D)
    ND = D // NDT
    # pre-tile weights into exact SBUF layouts (one contiguous run per
    # partition per DMA)
    w1t = (w1[c].astype(ml_dtypes.bfloat16).reshape(KD, 128, H)
           .transpose(1, 0, 2))                     # [128, KD, H]
    w1t = np.ascontiguousarray(
        np.stack([w1t[:, :, q * MHQ * 128:(q + 1) * MHQ * 128]
                  for q in range(NQ1)]))            # [NQ1,128,KD,MHQ*128]
    w2t = (w2[c].astype(ml_dtypes.bfloat16).reshape(MH, 128, D)
           .transpose(1, 0, 2))                     # [128, MH, D]
    w2t = np.ascontiguousarray(
        np.stack([w2t[:, :, nd * NDT:(nd + 1) * NDT]
                  for nd in range(ND)]))            # [ND,128,MH,NDT]
    return {
        "w1": w1t,
        "b1": np.ascontiguousarray(
            b1[c].astype(np.float32).reshape(MH, 128).T),
        "w2": w2t,
        "b2": np.ascontiguousarray(b2[c]).astype(np.float32),
        "gw": np.ascontiguousarray(
            gate_w[:, rot].astype(np.float32).reshape(KD, 128, E)
            .transpose(1, 0, 2)),
        "gb": np.ascontiguousarray(gate_b[rot]).astype(np.float32),
    }


def _prep_x(xg_cols, C):
    """xg_cols: [D, C] fp32 gathered x^T. Returns pre-tiled xg/xb arrays."""
    D = xg_cols.shape[0]
    KD = D // 128
    TC = C // 128
    v = xg_cols.reshape(KD, 128, C)
    xg_t = np.ascontiguousarray(
        v.reshape(KD, 128, TC, 128).transpose(2, 1, 0, 3))  # [TC,128,KD,128]
    xb_full = v.astype(ml_dtypes.bfloat16).transpose(1, 0, 2)  # [128, KD, C]
    widths = [NT_MAX] * (C // NT_MAX)
    if C % NT_MAX:
        widths.append(C % NT_MAX)
    blocks = []
    ts = 0
    for NT in widths:
        blocks.append(np.ascontiguousarray(xb_full[:, :, ts:ts + NT]).ravel())
        ts += NT
    xb_t = np.concatenate(blocks)
    return xg_t, xb_t


def run_moe(x, gate_w, gate_b, w1, b1, w2, b2, trace=False, trace_kwargs=None):
    """Sparse expert-parallel run. Returns ((combined, load_loss,
    expert_load), BassKernelResults)."""
    Bx, Sx, D = x.shape
    T = Bx * Sx
    H = w1.shape[2]

    xflat = x.reshape(T, D).astype(np.float32)
    # Host-side routing (sharding decision): which tokens go to which core.
    logits = xflat.astype(np.float64) @ gate_w.astype(np.float64) \
        + gate_b.astype(np.float64)
    top2 = np.argsort(-logits, axis=1)[:, :TOP_K]
    token_idx = [np.where((top2 == c).any(axis=1))[0] for c in range(E)]
    n_max = max(len(ix) for ix in token_idx)
    C = ((n_max + 127) // 128) * 128

    with_b2 = bool(np.any(b2))
    nc = _get_nc(C, D, H, with_b2, t_active=n_max)

    xT = np.ascontiguousarray(xflat.T)  # [D, T]
    in_maps = []
    for c in range(N_CORES):
        ix = token_idx[c]
        xg = np.zeros((D, C), dtype=np.float32)
        xg[:, :len(ix)] = xT[:, ix]
        vld = np.zeros(C, dtype=np.float32)
        vld[:len(ix)] = 1.0
        vld = np.ascontiguousarray(vld.reshape(C // 128, 128).T)
        m = _per_core_weights(c, w1, b1, w2, b2, gate_w, gate_b)
        m["xg"], m["xb"] = _prep_x(xg, C)
        m["valid"] = vld
        in_maps.append(m)

    res = _run_cores(nc, in_maps, trace, trace_kwargs)

    combined = np.zeros((T, D), dtype=np.float32)
    expert_load = np.zeros(E, dtype=np.float32)
    for c in range(N_CORES):
        ix = token_idx[c]
        np.add.at(combined, ix, res.results[c]["out"][:len(ix)])
        expert_load[c] = res.results[c]["loads"][0, 0] / np.float32(T)
    combined = combined.reshape(Bx, Sx, D)
    load_loss = np.float32(LB_W * np.sum((expert_load - 1.0 / E) ** 2))
    return (combined, load_loss, expert_load), res


def run_moe_dense(x, gate_w, gate_b, w1, b1, w2, b2, trace=False,
                  trace_kwargs=None):
    """Dense expert-parallel fallback: every core runs all T tokens."""
    Bx, Sx, D = x.shape
    T = Bx * Sx
    H = w1.shape[2]
    with_b2 = bool(np.any(b2))
    nc = _get_nc(T, D, H, with_b2)

    xT = np.ascontiguousarray(x.reshape(T, D).T.astype(np.float32))
    xg_t, xb_t = _prep_x(xT, T)
    ones = np.ones((128, T // 128), dtype=np.float32)

    in_maps = []
    for c in range(N_CORES):
        m = _per_core_weights(c, w1, b1, w2, b2, gate_w, gate_b)
        m["xg"] = xg_t
        m["xb"] = xb_t
        m["valid"] = ones
        in_maps.append(m)

    res = _run_cores(nc, in_maps, trace, trace_kwargs)

    combined = np.zeros((T, D), dtype=np.float32)
    for c in range(N_CORES):
        combined += res.results[c]["out"]
    combined = combined.reshape(Bx, Sx, D)
    expert_load = (res.results[0]["loads"][:, 0] / np.float32(T)).astype(np.float32)
    load_loss = np.float32(LB_W * np.sum((expert_load - 1.0 / E) ** 2))
    return (combined, load_loss, expert_load), res


def kernel(x, gate_w, gate_b, w1, b1, w2, b2):
    trace = bool(int(os.environ.get("BASS_MOE_TRACE", "0")))
    fn = run_moe_dense if os.environ.get("MOE_DENSE") else run_moe
    out, _ = fn(x, gate_w, gate_b, w1, b1, w2, b2, trace=trace)
    return out
